# revision 12
# baseline (speedup 1.0000x reference)
"""Chamfer distance matrix (L2) kernel for 8 Trainium2 NeuronCores.

Problem: xyz1 [B=32, G1=64, N1=32, 3], xyz2 [B=32, G2=64, N2=32, 3] ->
out[b, g1, g2] = mean_n1 min_n2 d + mean_n2 min_n1 d, where
d[n1, n2] = |x - y|^2 between points of group (b, g1) and (b, g2).

Strategy (data-parallel over B, 4 batches per core):
  - Host packs points into augmented 5-vectors so one K=5 matmul produces
    the full pairwise squared-distance matrix:
      X' = (|x|^2, 1, -2x),  Y' = (1, |y|^2, y),  d = X'.Y'
  - Per (batch, block of 4 groups): PE emits d as two [128, 1024] PSUM
    tiles (n2 split in halves) in float32r (1 cycle/row).
  - DVE min-tree: L1 = elementwise min of the two PSUM halves -> fp16
    SBUF, then 4 more fp16 (2x mode) min levels -> per-point
    nearest-neighbor distances [128 points, 64 groups].
  - PE computes the mean over the 32 points of each group with a tiny
    block-diagonal (1/32) matmul (partition-dim reduction).
  - Both orientations (min over n2, min over n1) run the same way with
    lhsT/rhs swapped; the second is transposed on PE and added on DVE.
"""

import functools
import numpy as np

import concourse.bass as bass
import concourse.tile as tile
from concourse import bacc, mybir
from concourse import bass_utils

F32 = mybir.dt.float32
F32R = mybir.dt.float32r
F16 = mybir.dt.float16
MIN = mybir.AluOpType.min

B, G, N = 32, 64, 32
NCORES = 8
BPC = B // NCORES          # batches per core
PTS = BPC * G * N          # points per core per set (8192)
GBLK = G // 4              # 16 blocks of 4 groups

# Set by test.py to collect an NTFF profile + exec time.
TRACE = False
TRACE_DIR = None
LAST_EXEC_NS = None
LAST_RESULT = None


def _min_tree(nc, pools, dh0, dh1):
    """Min-reduction over n2 (innermost 16+16 of the two halves).

    dh0/dh1: [128, 1024] f32 PSUM tiles, columns = (64 groups, 16 half-pts).
    Walrus rejects TensorTensor with two PSUM operands, so ACT first stages
    dh1 into SBUF (it is otherwise idle), then DVE runs the min tree.
    Returns [128, 64] fp16 SBUF tile of per-(point, group) min distances.
    """
    s1pool, t1pool, t2pool, t3pool, t4pool, mpool = pools
    sb1 = s1pool.tile([128, 1024], F32)
    nc.scalar.copy(sb1[:], dh1[:])
    t1 = t1pool.tile([128, 1024], F16)
    nc.vector.tensor_tensor(t1[:], dh0[:], sb1[:], op=MIN)
    t1v = t1[:].rearrange("p (g n) -> p g n", n=16)
    t2 = t2pool.tile([128, 512], F16)
    t2v = t2[:].rearrange("p (g n) -> p g n", n=8)
    nc.vector.tensor_tensor(t2v, t1v[:, :, 0:8], t1v[:, :, 8:16], op=MIN)
    t3 = t3pool.tile([128, 256], F16)
    t3v = t3[:].rearrange("p (g n) -> p g n", n=4)
    nc.vector.tensor_tensor(t3v, t2v[:, :, 0:4], t2v[:, :, 4:8], op=MIN)
    t4 = t4pool.tile([128, 128], F16)
    t4v = t4[:].rearrange("p (g n) -> p g n", n=2)
    nc.vector.tensor_tensor(t4v, t3v[:, :, 0:2], t3v[:, :, 2:4], op=MIN)
    m = mpool.tile([128, 64], F16)
    nc.vector.tensor_tensor(m[:], t4v[:, :, 0], t4v[:, :, 1], op=MIN)
    return m


@functools.lru_cache(maxsize=1)
def _build():
    nc = bacc.Bacc(
        "TRN2", target_bir_lowering=False, debug=False, enable_asserts=False
    )
    xa_d = nc.dram_tensor("xa", [5, PTS], F16, kind="ExternalInput")
    ya_d = nc.dram_tensor("ya", [5, PTS], F16, kind="ExternalInput")
    bo_d = nc.dram_tensor("bo", [128, 4], F16, kind="ExternalInput")
    id_d = nc.dram_tensor("id64", [64, 64], F32, kind="ExternalInput")
    out_d = nc.dram_tensor("out", [BPC, G, G], F32, kind="ExternalOutput")

    with tile.TileContext(nc) as tc:
        with (
            tc.tile_pool(name="const", bufs=1) as cpool,
            tc.tile_pool(name="dpsum", bufs=3, space="PSUM") as dpool,
            tc.tile_pool(name="zpsum", bufs=2, space="PSUM") as zpool,
            tc.tile_pool(name="s1", bufs=2) as s1pool,
            tc.tile_pool(name="t1", bufs=2) as t1pool,
            tc.tile_pool(name="t2", bufs=2) as t2pool,
            tc.tile_pool(name="t3", bufs=2) as t3pool,
            tc.tile_pool(name="t4", bufs=2) as t4pool,
            tc.tile_pool(name="m", bufs=2) as mpool,
            tc.tile_pool(name="acc", bufs=1) as apool,
        ):
            XA = cpool.tile([5, PTS], F16)
            nc.sync.dma_start(XA[:], xa_d.ap()[:])
            YA = cpool.tile([5, PTS], F16)
            nc.sync.dma_start(YA[:], ya_d.ap()[:])
            BO = cpool.tile([128, 4], F16)
            nc.sync.dma_start(BO[:], bo_d.ap()[:])
            ID = cpool.tile([64, 64], F32)
            nc.sync.dma_start(ID[:], id_d.ap()[:])

            # OA holds per-b [64 g2, 64 g1] (A side, transposed); OB holds
            # per-b [64 g1, 64 g2]. OC = OB + transpose(OA).
            OA = apool.tile([64, BPC * 64], F32)
            OB = apool.tile([64, BPC * 64], F32)
            OC = apool.tile([64, BPC * 64], F32)

            tree_pools = (s1pool, t1pool, t2pool, t3pool, t4pool, mpool)

            def side(b, i, L, R, Z):
                # lhsT: the 4 stationary groups (128 points) of batch b.
                lv = L[:].rearrange("k (b p) -> k b p", b=BPC)
                lhsT = lv[:, b, i * 128 : (i + 1) * 128]
                # rhs: all 64 moving groups of batch b, n split in halves.
                rv = R[:].rearrange("k (b g n) -> k b g n", b=BPC, g=G)
                dh = []
                for h in range(2):
                    t = dpool.tile([128, 1024], F32, tag="d")
                    for k in range(2):
                        rhs = rv[
                            :, b, 32 * k : 32 * (k + 1), 16 * h : 16 * (h + 1)
                        ]
                        nc.tensor.matmul(
                            t[:, 512 * k : 512 * (k + 1)],
                            lhsT,
                            rhs,
                            start=True,
                            stop=True,
                        )
                    dh.append(t)
                m = _min_tree(nc, tree_pools, dh[0], dh[1])
                # Mean over the 32 points of each stationary group:
                # out[g, blk] = sum_k m[k, g] * bo[k, blk] (bo = blockdiag 1/32).
                nc.tensor.matmul(
                    Z[:, 4 * i : 4 * (i + 1)], m[:], BO[:], start=True, stop=True
                )

            for b in range(BPC):
                zA = zpool.tile([64, 64], F32, tag="z")
                for i in range(GBLK):
                    side(b, i, XA, YA, zA)
                nc.scalar.copy(OA[:, 64 * b : 64 * (b + 1)], zA[:])
                zB = zpool.tile([64, 64], F32, tag="z")
                for j in range(GBLK):
                    side(b, j, YA, XA, zB)
                nc.scalar.copy(OB[:, 64 * b : 64 * (b + 1)], zB[:])

            for b in range(BPC):
                zt = zpool.tile([64, 64], F32, tag="z")
                nc.tensor.matmul(
                    zt[:],
                    OA[:, 64 * b : 64 * (b + 1)],
                    ID[:],
                    is_transpose=True,
                    start=True,
                    stop=True,
                )
                nc.vector.tensor_add(
                    OC[:, 64 * b : 64 * (b + 1)], OB[:, 64 * b : 64 * (b + 1)], zt[:]
                )

            ov = out_d.ap().rearrange("b g h -> g b h")
            ocv = OC[:].rearrange("p (b h) -> p b h", b=BPC)
            nc.sync.dma_start(ov, ocv)

    nc.compile()
    return nc


def _host_prep(xyz1, xyz2):
    x = np.ascontiguousarray(xyz1, dtype=np.float32).reshape(B * G * N, 3)
    y = np.ascontiguousarray(xyz2, dtype=np.float32).reshape(B * G * N, 3)
    xa = np.empty((5, B * G * N), np.float16)
    xa[0] = (x * x).sum(-1)
    xa[1] = 1.0
    xa[2:5] = -2.0 * x.T
    ya = np.empty((5, B * G * N), np.float16)
    ya[0] = 1.0
    ya[1] = (y * y).sum(-1)
    ya[2:5] = y.T
    bo = np.zeros((128, 4), np.float16)
    for mblk in range(4):
        bo[32 * mblk : 32 * (mblk + 1), mblk] = 1.0 / 32
    id64 = np.eye(64, dtype=np.float32)
    return xa, ya, bo, id64


def kernel(xyz1_matrix, xyz2_matrix):
    global LAST_EXEC_NS
    xa, ya, bo, id64 = _host_prep(np.asarray(xyz1_matrix), np.asarray(xyz2_matrix))
    nc = _build()
    in_maps = []
    for c in range(NCORES):
        sl = slice(c * PTS, (c + 1) * PTS)
        in_maps.append(
            {
                "xa": np.ascontiguousarray(xa[:, sl]),
                "ya": np.ascontiguousarray(ya[:, sl]),
                "bo": bo,
                "id64": id64,
            }
        )
    global LAST_RESULT
    res = bass_utils.run_bass_kernel_spmd(
        nc, in_maps, core_ids=list(range(NCORES)), trace=TRACE, tmpdir=TRACE_DIR
    )
    LAST_RESULT = res
    LAST_EXEC_NS = res.exec_time_ns
    out = np.concatenate([r["out"] for r in res.results], axis=0)
    return out.astype(np.float32)
